# revision 7
# baseline (speedup 1.0000x reference)
"""Expert-parallel MoE routing kernel for Trainium2 (8 NeuronCores).

Problem: top-k(=2) softmax-gated MoE FFN (relu), followed by
log_softmax(sum(moe_out, axis=-1)) over the sequence dim.

Key algebraic observation: the graded output is
    log_softmax_S( sum_d moe_out[t, d] )
and
    sum_d moe_out[t, :] = sum_e combine[t,e] * (relu(x_t @ W1_e + b1_e) @ rowsum(W2_e) + sum(b2_e))
so the second expert matmul collapses to a matvec against rowsum(W2_e).
All of W2 must still be read from HBM (memory-regime roofline unchanged);
its row-sum is computed on-device by the Vector engine while W1 streams
into the Tensor engine.

Sharding (per the expert-parallel hint): core e owns expert e's weights.
The host computes the (tiny) gate/top-k routing to build the dispatch
(it must, to construct the per-core input shards), gathers each expert's
tokens, and the device does the entire FFN including gate-value scaling.
Host then scatter-adds the per-(token,expert) scalars and applies the
final log_softmax on the [B, S] result.

Matmuls run as float32r (fp32 storage, reduced-precision PE mode, 4x the
throughput of strict fp32; measured rel-err ~2e-4 end to end).
"""

import os

import numpy as np

N_CORES = 8
P = 128


def _round_up(v, m):
    return ((v + m - 1) // m) * m


def _chunks(C):
    # rhs free-dim chunks; keep each >=256 so float32r runs at 1 cycle/row
    if C <= 512:
        return [(0, C)]
    assert C % 256 == 0
    h = C // 2
    return [(0, h), (h, h)]


_BUILD_CACHE = {}

# matmul operand dtype: "bf16" (1 cy/row, in-flight cast on DMA, ~3e-3 rel err)
# or "f32r" (fp32 storage, ~2.8 cy/row, ~1e-4 rel err)
MM_MODE = os.environ.get("MOE_MM_MODE", "bf16")


def _build_program(D, H, C, n_b2):
    """Trace + compile the single-core program (SPMD across 8 cores).

    Per-core inputs:
      xtg [D, C]  f32r  gathered tokens for this expert, transposed
      w1  [D, H]  f32r  expert's first-layer weight (natural = lhsT layout)
      b1t [P, H/P] f32  expert's first-layer bias, column m = b1[m*128:(m+1)*128]
      w2  [H, n_b2] f32 expert's second-layer weight
      b2  [1, n_b2] f32 expert's second-layer bias
      g   [1, C]  f32   gate values per slot (0 for padding slots)
    Output:
      z [1, C] f32 = g * (relu(x @ w1 + b1) @ rowsum(w2) + sum(b2))
    """
    key = (D, H, C, n_b2, MM_MODE)
    if key in _BUILD_CACHE:
        return _BUILD_CACHE[key]

    import concourse.tile as tile
    from concourse import bacc, mybir

    f32 = mybir.dt.float32
    mmdt = mybir.dt.bfloat16 if MM_MODE == "bf16" else mybir.dt.float32r
    KD = D // P  # k-tiles over D
    MH = H // P  # m-tiles over H
    chunks = _chunks(C)

    nc = bacc.Bacc("TRN2", target_bir_lowering=False, debug=False)
    in_dt = f32 if MM_MODE == "bf16" else mybir.dt.float32r
    xtg_d = nc.dram_tensor("xtg", [D, C], in_dt, kind="ExternalInput").ap()
    w1_d = nc.dram_tensor("w1", [D, H], in_dt, kind="ExternalInput").ap()
    b1t_d = nc.dram_tensor("b1t", [P, MH], f32, kind="ExternalInput").ap()
    w2_d = nc.dram_tensor("w2", [H, n_b2], f32, kind="ExternalInput").ap()
    b2_d = nc.dram_tensor("b2", [1, n_b2], f32, kind="ExternalInput").ap()
    g_d = nc.dram_tensor("g", [1, C], f32, kind="ExternalInput").ap()
    z_d = nc.dram_tensor("z", [1, C], f32, kind="ExternalOutput").ap()

    def dma_in(out_ap, in_ap):
        # gpsimd DMA casts in flight when dtypes differ; sync DMA otherwise
        if out_ap.dtype != in_ap.dtype:
            nc.gpsimd.dma_start(out=out_ap, in_=in_ap)
        else:
            nc.sync.dma_start(out=out_ap, in_=in_ap)

    with tile.TileContext(nc) as tc:
        with (
            tc.tile_pool(name="persist", bufs=1) as persist,
            tc.tile_pool(name="w2p", bufs=3) as w2p,
            tc.tile_pool(name="psum_h", bufs=4, space="PSUM") as psum_h,
            tc.tile_pool(name="psum_z", bufs=2, space="PSUM") as psum_z,
        ):
            # --- small persistent loads ---
            g_sb = persist.tile([1, C], f32)
            nc.sync.dma_start(out=g_sb[:], in_=g_d[:])
            b1t_sb = persist.tile([P, MH], f32)
            nc.sync.dma_start(out=b1t_sb[:], in_=b1t_d[:])
            b2_sb = persist.tile([1, n_b2], f32)
            nc.sync.dma_start(out=b2_sb[:], in_=b2_d[:])
            b2sum = persist.tile([1, 1], f32)
            nc.vector.reduce_sum(out=b2sum[:], in_=b2_sb[:], axis=mybir.AxisListType.X)

            # --- activations, transposed+gathered: 8 tiles [128, C] ---
            xtg_tiles = []
            for kd in range(KD):
                t = persist.tile([P, C], mmdt, tag=f"xtg{kd}", name=f"xtg{kd}")
                dma_in(t[:], xtg_d[kd * P : (kd + 1) * P, :])
                xtg_tiles.append(t)

            # --- w1: resident k-major tiles, one big DMA each ---
            w1_tiles = []
            for kd in range(KD):
                t = persist.tile([P, H], mmdt, tag=f"w1k{kd}", name=f"w1k{kd}")
                dma_in(t[:], w1_d[kd * P : (kd + 1) * P, :])
                w1_tiles.append(t)

            # w2 row-sums, column m = rowsum over free dim of w2 m-tile
            w2sum = persist.tile([P, MH], mmdt)
            # hT per chunk: [128, MH * chunk_len], slice m holds h^T m-tile
            ht_tiles = [
                persist.tile([P, MH * ln], mmdt, tag=f"ht{ci}", name=f"ht{ci}")
                for ci, (off, ln) in enumerate(chunks)
            ]

            # --- w2 stream (DVE row-sums, overlaps the PE loop) ---
            for m in range(MH):
                w2t = w2p.tile([P, n_b2], f32)
                nc.sync.dma_start(out=w2t[:], in_=w2_d[m * P : (m + 1) * P, :])
                with nc.allow_low_precision(
                    reason="w2 row-sum feeds reduced-precision matmul anyway"
                ):
                    nc.vector.reduce_sum(
                        out=w2sum[:, m : m + 1], in_=w2t[:], axis=mybir.AxisListType.X
                    )

            # --- mm1 + relu: h^T[m-tile] = relu(w1^T x^T + b1) ---
            for ci, (off, ln) in enumerate(chunks):
                for m in range(MH):
                    ps = psum_h.tile([P, ln], f32)
                    for kd in range(KD):
                        nc.tensor.matmul(
                            ps[:],
                            w1_tiles[kd][:, m * P : (m + 1) * P],
                            xtg_tiles[kd][:, off : off + ln],
                            start=(kd == 0),
                            stop=(kd == KD - 1),
                        )
                    nc.scalar.activation(
                        ht_tiles[ci][:, m * ln : (m + 1) * ln],
                        ps[:],
                        mybir.ActivationFunctionType.Relu,
                        bias=b1t_sb[:, m : m + 1],
                    )

            # --- matvec against w2 row-sums + bias + gate scale ---
            z_sb = persist.tile([1, C], f32)
            for ci, (off, ln) in enumerate(chunks):
                pz = psum_z.tile([1, ln], f32)
                for m in range(MH):
                    nc.tensor.matmul(
                        pz[:],
                        w2sum[:, m : m + 1],
                        ht_tiles[ci][:, m * ln : (m + 1) * ln],
                        start=(m == 0),
                        stop=(m == MH - 1),
                    )
                nc.scalar.activation(
                    z_sb[:, off : off + ln],
                    pz[:],
                    mybir.ActivationFunctionType.Identity,
                    bias=b2sum[:],
                )
                nc.vector.tensor_mul(
                    z_sb[:, off : off + ln],
                    z_sb[:, off : off + ln],
                    g_sb[:, off : off + ln],
                )
            nc.sync.dma_start(out=z_d[:], in_=z_sb[:])

    nc.compile()
    _BUILD_CACHE[key] = nc
    return nc


def kernel(x, wg, w1, b1, w2, b2, k):
    from concourse.bass_utils import run_bass_kernel_spmd

    x = np.asarray(x)
    wg = np.asarray(wg)
    w1 = np.asarray(w1)
    b1 = np.asarray(b1)
    w2 = np.asarray(w2)
    b2 = np.asarray(b2)
    k = int(k)

    B, S, D = x.shape
    E = wg.shape[1]
    H = w1.shape[2]
    T = B * S
    assert E == N_CORES, f"expert-parallel layout assumes E == 8, got {E}"

    xf = np.ascontiguousarray(x.reshape(T, D), dtype=np.float32)

    # --- gate + top-k routing (host; needed to build the dispatch shards) ---
    logits = xf @ wg.astype(np.float32)
    logits -= logits.max(axis=1, keepdims=True)
    np.exp(logits, out=logits)
    scores = logits / logits.sum(axis=1, keepdims=True)
    if k >= E:
        topi = np.broadcast_to(np.arange(E, dtype=np.int64), (T, E))
    else:
        topi = np.argpartition(-scores, k, axis=1)[:, :k]
    rows = np.arange(T)[:, None]
    topv = scores[rows, topi]

    # per-expert token lists
    idx_e = []
    val_e = []
    for e in range(E):
        tmask, kpos = np.nonzero(topi == e)
        idx_e.append(tmask)
        val_e.append(topv[tmask, kpos].astype(np.float32))
    max_cnt = max(len(i) for i in idx_e)
    C = max(512, _round_up(max_cnt, 256))

    nc = _build_program(D, H, C, w2.shape[2])

    in_maps = []
    for e in range(E):
        n_e = len(idx_e[e])
        xtg = np.zeros((D, C), dtype=np.float32)
        xtg[:, :n_e] = xf[idx_e[e]].T
        g = np.zeros((1, C), dtype=np.float32)
        g[0, :n_e] = val_e[e]
        b1t = np.ascontiguousarray(
            b1[e].astype(np.float32).reshape(H // P, P).T
        )
        in_maps.append(
            {
                "xtg": xtg,
                "w1": np.ascontiguousarray(w1[e], dtype=np.float32),
                "b1t": b1t,
                "w2": np.ascontiguousarray(w2[e], dtype=np.float32),
                "b2": np.ascontiguousarray(b2[e][None, :], dtype=np.float32),
                "g": g,
            }
        )

    res = run_bass_kernel_spmd(nc, in_maps, core_ids=list(range(N_CORES)))

    # --- combine: scatter-add per-(token, expert) scalars, then log_softmax ---
    s = np.zeros(T, dtype=np.float32)
    for e in range(E):
        n_e = len(idx_e[e])
        if n_e:
            s[idx_e[e]] += res.results[e]["z"][0, :n_e]

    sm = s.reshape(B, S)
    sm = sm - sm.max(axis=1, keepdims=True)
    out = sm - np.log(np.exp(sm).sum(axis=1, keepdims=True))
    return out.astype(np.float32)
